# revision 3
# baseline (speedup 1.0000x reference)
"""Trainium2 Bass kernel for MessageGraphConvolution.

out = (segment_mean of x[src] over dst) @ W.T + x @ B.T

Strategy (8 NeuronCores, SPMD):
  - Pad nodes 10000 -> 10240 = 80 blocks of 128. Core c owns dst blocks
    [10c, 10c+10) (edge-parallel sharding by destination block, so no
    cross-core reduction is needed).
  - Host prep: route each edge to its dst block, pad each block's edge
    list to C chunks of 128 edges (pad: src=0, dst_local=-1).
  - Device, per block: dma_gather 128-edge chunks of x16 (fp16 copy of x,
    cast on device) -> messages M [128e, 128f]; build one-hot
    P [128e, 128d] = (iota == dst_local) on DVE; accumulate in PSUM via
    matmul P.T @ [M | 1] -> [128d, 128f agg | deg]; normalize by
    max(deg,1) reciprocal; PE-transpose; two fp32 GEMMs (agg@W.T + x@B.T).
"""

import numpy as np

NUM_NODES = 10000
IN_FEAT = 128
OUT_FEAT = 128
N_CORES = 8
BLK = 128
NODES_PAD = 10240          # 80 blocks of 128
NBLK_TOTAL = NODES_PAD // BLK   # 80
BLK_PER_CORE = NBLK_TOTAL // N_CORES  # 10
NODES_PER_CORE = BLK_PER_CORE * BLK   # 1280


def _wrap_idx(ids):
    """Edge-order int16 ids [SLOTS] -> dma_gather SBUF layout [128, SLOTS//16].

    Empirically decoded on HW: output slot (partition p, chunk c) reads the
    index stored at wrapped[p % 16, c*8 + p//16]; the [16, n/16] wrap is
    replicated 8x across partition groups.
    """
    w = ids.reshape(-1, 8, 16).transpose(2, 0, 1).reshape(16, -1)
    return np.tile(w, (8, 1))


def _build_program(C, reps=1):
    import concourse.bacc as bacc
    import concourse.mybir as mybir
    from concourse.tile import TileContext

    SLOTS = C * BLK
    MW = 132  # 128 feats + ones col + 3 pad (fp16 alignment)

    nc = bacc.Bacc("TRN2")
    xf_t = nc.dram_tensor("xf", [NODES_PAD, IN_FEAT], mybir.dt.float32, kind="ExternalInput")
    xc_t = nc.dram_tensor("xc", [NODES_PER_CORE, IN_FEAT], mybir.dt.float32, kind="ExternalInput")
    idx_t = nc.dram_tensor("idx", [128, BLK_PER_CORE * SLOTS // 16], mybir.dt.int16, kind="ExternalInput")
    dstT_t = nc.dram_tensor("dstT", [128, BLK_PER_CORE * C], mybir.dt.float32, kind="ExternalInput")
    iota_t = nc.dram_tensor("iota", [128, 128], mybir.dt.float16, kind="ExternalInput")
    ident_t = nc.dram_tensor("ident", [128, 128], mybir.dt.float32, kind="ExternalInput")
    W_t = nc.dram_tensor("W", [OUT_FEAT, IN_FEAT], mybir.dt.float32, kind="ExternalInput")
    B_t = nc.dram_tensor("Bm", [OUT_FEAT, IN_FEAT], mybir.dt.float32, kind="ExternalInput")
    out_t = nc.dram_tensor("out", [NODES_PER_CORE, OUT_FEAT], mybir.dt.float32, kind="ExternalOutput")

    with TileContext(nc) as tc:
        with tc.tile_pool(name="dram", bufs=1, space="DRAM") as dpool:
            x16_t = dpool.tile([NODES_PAD, IN_FEAT], mybir.dt.float16)
            for _rep in range(reps):
                # ---------- prologue: cast x -> fp16 in DRAM ----------
                with tc.tile_pool(name="cast", bufs=2) as cpool:
                    xf_flat = xf_t[:].rearrange("(p k) f -> p (k f)", p=128)
                    x16_flat = x16_t[:].rearrange("(p k) f -> p (k f)", p=128)
                    KTOT = NODES_PAD * IN_FEAT // 128  # 10240
                    NSL = 4
                    KS = KTOT // NSL
                    for s in range(NSL):
                        tf = cpool.tile([128, KS], mybir.dt.float32, tag="tf")
                        t16 = cpool.tile([128, KS], mybir.dt.float16, tag="t16")
                        nc.sync.dma_start(tf[:], xf_flat[:, s * KS:(s + 1) * KS])
                        if s % 2 == 0:
                            nc.scalar.copy(t16[:], tf[:])
                        else:
                            nc.vector.tensor_copy(t16[:], tf[:])
                        nc.sync.dma_start(x16_flat[:, s * KS:(s + 1) * KS], t16[:])

                # ---------- constants ----------
                with tc.tile_pool(name="const", bufs=1) as const, \
                     tc.tile_pool(name="gpool", bufs=2) as gpool, \
                     tc.tile_pool(name="mpool", bufs=2) as mpool, \
                     tc.tile_pool(name="ppool", bufs=6) as ppool, \
                     tc.tile_pool(name="epool", bufs=2) as epool, \
                     tc.tile_pool(name="psA", bufs=2, space="PSUM") as psA, \
                     tc.tile_pool(name="psT", bufs=2, space="PSUM") as psT, \
                     tc.tile_pool(name="psO", bufs=2, space="PSUM") as psO:
                    iota = const.tile([128, 128], mybir.dt.float16)
                    ident = const.tile([128, 128], mybir.dt.float32)
                    Wsb = const.tile([128, 128], mybir.dt.float32)
                    Bsb = const.tile([128, 128], mybir.dt.float32)
                    WT = const.tile([128, 128], mybir.dt.float32)
                    BT = const.tile([128, 128], mybir.dt.float32)
                    idxs = const.tile([128, BLK_PER_CORE * SLOTS // 16], mybir.dt.int16)
                    dstT = const.tile([128, BLK_PER_CORE * C], mybir.dt.float32)
                    nc.sync.dma_start(iota[:], iota_t[:])
                    nc.sync.dma_start(ident[:], ident_t[:])
                    nc.sync.dma_start(Wsb[:], W_t[:])
                    nc.sync.dma_start(Bsb[:], B_t[:])
                    nc.sync.dma_start(idxs[:], idx_t[:])
                    nc.sync.dma_start(dstT[:], dstT_t[:])
                    pW = psT.tile([128, 128], mybir.dt.float32, tag="pt")
                    nc.tensor.transpose(pW[:], Wsb[:], ident[:])
                    nc.vector.tensor_copy(WT[:], pW[:])
                    pB = psT.tile([128, 128], mybir.dt.float32, tag="pt")
                    nc.tensor.transpose(pB[:], Bsb[:], ident[:])
                    nc.vector.tensor_copy(BT[:], pB[:])

                    # ---------- main loop over this core's 10 blocks ----------
                    for b in range(BLK_PER_CORE):
                        g = gpool.tile([128, C, 128], mybir.dt.float16, tag="g")
                        nc.gpsimd.dma_gather(
                            g[:], x16_t[:],
                            idxs[:, b * (SLOTS // 16):(b + 1) * (SLOTS // 16)],
                            SLOTS, SLOTS, 128, single_packet=False)
                        m = mpool.tile([128, C, MW], mybir.dt.float16, tag="m")
                        nc.vector.memset(m[:, :, 128:129], 1.0)
                        nc.scalar.copy(m[:, :, 0:128], g[:])
                        pa = psA.tile([128, 129], mybir.dt.float32, tag="pa")
                        for c in range(C):
                            P = ppool.tile([128, 128], mybir.dt.float16, tag="P")
                            nc.vector.tensor_scalar(
                                P[:], iota[:], dstT[:, b * C + c:b * C + c + 1],
                                None, mybir.AluOpType.is_equal)
                            nc.tensor.matmul(pa[:], P[:], m[:, c, 0:129],
                                             start=(c == 0), stop=(c == C - 1))
                        # ---------- epilogue ----------
                        degc = epool.tile([128, 1], mybir.dt.float32, tag="degc")
                        recip = epool.tile([128, 1], mybir.dt.float32, tag="recip")
                        nc.vector.tensor_scalar(degc[:], pa[:, 128:129], 1.0,
                                                None, mybir.AluOpType.max)
                        nc.vector.reciprocal(recip[:], degc[:])
                        nagg = epool.tile([128, 128], mybir.dt.float32, tag="nagg")
                        nc.vector.tensor_scalar(nagg[:], pa[:, 0:128], recip[:],
                                                None, mybir.AluOpType.mult)
                        pt = psT.tile([128, 128], mybir.dt.float32, tag="pt")
                        nc.tensor.transpose(pt[:], nagg[:], ident[:])
                        aggT = epool.tile([128, 128], mybir.dt.float32, tag="aggT")
                        nc.vector.tensor_copy(aggT[:], pt[:])
                        xb = epool.tile([128, 128], mybir.dt.float32, tag="xb")
                        nc.sync.dma_start(xb[:], xc_t[b * 128:(b + 1) * 128, :])
                        pt2 = psT.tile([128, 128], mybir.dt.float32, tag="pt")
                        nc.tensor.transpose(pt2[:], xb[:], ident[:])
                        xT = epool.tile([128, 128], mybir.dt.float32, tag="xT")
                        nc.vector.tensor_copy(xT[:], pt2[:])
                        po = psO.tile([128, 128], mybir.dt.float32, tag="po")
                        nc.tensor.matmul(po[:], aggT[:], WT[:], start=True, stop=False)
                        nc.tensor.matmul(po[:], xT[:], BT[:], start=False, stop=True)
                        osb = epool.tile([128, 128], mybir.dt.float32, tag="osb")
                        nc.vector.tensor_copy(osb[:], po[:])
                        nc.sync.dma_start(out_t[b * 128:(b + 1) * 128, :], osb[:])
    nc.compile()
    return nc


def _host_prep(x, edge_index):
    src = np.asarray(edge_index[0], dtype=np.int64)
    dst = np.asarray(edge_index[1], dtype=np.int64)
    assert NUM_NODES <= 32767, "int16 gather indices"

    blk = (dst // BLK).astype(np.int64)
    order = np.argsort(blk, kind="stable")
    src_o = src[order].astype(np.int16)
    dstl_o = (dst[order] - blk[order] * BLK).astype(np.float32)
    cnt = np.bincount(blk, minlength=NBLK_TOTAL)
    C = max(1, int(np.ceil(cnt.max() / BLK)))
    SLOTS = C * BLK

    src_pad = np.zeros((NBLK_TOTAL, SLOTS), dtype=np.int16)
    dstl_pad = np.full((NBLK_TOTAL, SLOTS), -1.0, dtype=np.float32)
    off = 0
    for b in range(NBLK_TOTAL):
        n = int(cnt[b])
        src_pad[b, :n] = src_o[off:off + n]
        dstl_pad[b, :n] = dstl_o[off:off + n]
        off += n

    x_pad = np.zeros((NODES_PAD, IN_FEAT), dtype=np.float32)
    x_pad[:NUM_NODES] = x

    iota_host = np.tile(np.arange(128, dtype=np.float16), (128, 1))
    ident_host = np.eye(128, dtype=np.float32)

    per_core = []
    for core in range(N_CORES):
        bs = range(core * BLK_PER_CORE, (core + 1) * BLK_PER_CORE)
        idx_host = np.concatenate([_wrap_idx(src_pad[b]) for b in bs], axis=1)
        dstT_host = np.empty((128, BLK_PER_CORE * C), dtype=np.float32)
        for j, b in enumerate(bs):
            dstT_host[:, j * C:(j + 1) * C] = (
                dstl_pad[b].reshape(C, 128).T)
        xc = x_pad[core * NODES_PER_CORE:(core + 1) * NODES_PER_CORE]
        per_core.append({
            "xf": x_pad, "xc": np.ascontiguousarray(xc),
            "idx": idx_host, "dstT": dstT_host,
            "iota": iota_host, "ident": ident_host,
        })
    return C, per_core


def kernel(x, edge_index, W, B, _reps=1, _return_time=False):
    import time
    from concourse.bass_utils import run_bass_kernel_spmd

    x = np.asarray(x, dtype=np.float32)
    W = np.asarray(W, dtype=np.float32)
    B = np.asarray(B, dtype=np.float32)

    C, per_core = _host_prep(x, edge_index)
    nc = _build_program(C, reps=_reps)
    in_maps = []
    for core in range(N_CORES):
        m = dict(per_core[core])
        m["W"] = W
        m["Bm"] = B
        in_maps.append(m)

    t0 = time.monotonic()
    res = run_bass_kernel_spmd(nc, in_maps, core_ids=list(range(N_CORES)))
    wall = time.monotonic() - t0

    out = np.concatenate([np.asarray(r["out"]) for r in res.results], axis=0)
    out = out[:NUM_NODES].astype(np.float32)
    if _return_time:
        return out, wall
    return out
